# revision 3
# baseline (speedup 1.0000x reference)
"""Fourier-KAN autoencoder on 8 TRN2 NeuronCores (Bass/Tile).

Data-parallel over the batch dim (2048 rows/core, weights replicated).
Activations live transposed in SBUF as [feature, batch]. Per layer:
sin/cos of the range-reduced base angle come from the ACT Sin spline
(accurate on [-pi,pi]; cos via half-angle identity), harmonics 2..8 from
Chebyshev recurrences on DVE, and the (i,h)-contraction runs as fp32
matmuls on PE against weights pre-transposed on-chip (PE transpose) into
a K-major DRAM scratch.
"""

import math
import sys

import numpy as np

sys.path.insert(0, "/opt/trn_rl_repo")

import concourse.bass as bass
import concourse.mybir as mybir
import concourse.tile as tile
from concourse.bass_utils import run_bass_kernel_spmd
from concourse.masks import make_identity

F32 = mybir.dt.float32
AF = mybir.ActivationFunctionType
OP = mybir.AluOpType

N_CORES = 8
NS = 16384 // N_CORES  # 2048 rows per core
H = 8
TWO_PI = 2.0 * math.pi

LAYERS = [
    ("enc0_c", 256, 512, None),
    ("enc1_c", 512, 512, None),
    ("enc2_c", 512, 512, None),
    ("enc3_c", 512, 128, None),
    ("dec_c", 128, 256, None),
    ("pred0_c", 128, 512, "pred0_b"),
    ("pred1_c", 512, 64, "pred1_b"),
]

_cnt = [0]


def _split_waits(nc):
    """This walrus build accepts one sem-wait per instruction: move extra
    waits onto same-engine NOPs inserted just before the instruction."""
    for f in nc.m.functions:
        for bb in f.blocks:
            insts = bb.instructions
            changed = False
            new = []
            for inst in insts:
                si = inst.sync_info
                if si is not None:
                    waits = list(si.on_wait)
                    if len(waits) > 1:
                        for w in waits[:-1]:
                            nop = mybir.InstNoOp(name=f"I-wsplit-{_cnt[0]}")
                            _cnt[0] += 1
                            nop.engine = inst.engine
                            nop.sync_info = mybir.SyncInfo(on_wait=[w], on_update=[])
                            new.append(nop)
                        inst.sync_info = mybir.SyncInfo(
                            on_wait=[waits[-1]], on_update=list(si.on_update)
                        )
                        changed = True
                new.append(inst)
            if changed:
                bb.instructions = new


def _build():
    nc = bass.Bass()
    f_d = nc.dram_tensor("features", [NS, 256], F32, kind="ExternalInput")
    c_d = {}
    for name, I, O, _ in LAYERS:
        c_d[name] = nc.dram_tensor(name, [2, O, I, H], F32, kind="ExternalInput")
    b_d = {
        "pred0_b": nc.dram_tensor("pred0_b", [1, 512], F32, kind="ExternalInput"),
        "pred1_b": nc.dram_tensor("pred1_b", [1, 64], F32, kind="ExternalInput"),
    }
    out_d = nc.dram_tensor("out_o", [NS, 256], F32, kind="ExternalOutput")
    pred_d = nc.dram_tensor("pred_o", [NS, 64], F32, kind="ExternalOutput")

    with tile.TileContext(nc) as tc:
        with (
            tc.tile_pool(name="singles", bufs=1) as singles,
            tc.tile_pool(name="acts", bufs=2) as acts,
            tc.tile_pool(name="wraw", bufs=2) as wraw,
            tc.tile_pool(name="wstage", bufs=3) as wstage,
            tc.tile_pool(name="win", bufs=1) as win,
            tc.tile_pool(name="trig", bufs=1) as trigp,
            tc.tile_pool(name="ring", bufs=4) as ring,
            tc.tile_pool(name="outs", bufs=1) as outsp,
            tc.tile_pool(name="stage", bufs=3) as stagep,
            tc.tile_pool(name="dram", bufs=1, space="DRAM") as dram,
            tc.tile_pool(name="pst", bufs=2, space="PSUM") as pst,
            tc.tile_pool(name="psmm", bufs=1, space="PSUM") as psmm,
        ):
            ident = singles.tile([128, 128], F32)
            make_identity(nc, ident)

            # ---- weight transpose prologue: c[t,o,i,h] -> scratch[ci][i128][(t,h,o)]
            wt_scr = {}
            for name, I, O, _ in LAYERS:
                CI, OC = I // 128, (O + 127) // 128
                scr = dram.tile([CI, 128, 2 * H * O], F32, name=f"wt_{name}")
                wt_scr[name] = scr
                for t in range(2):
                    for oc in range(OC):
                        om = min(128, O - oc * 128)
                        for ci in range(CI):
                            raw = wraw.tile([128, 128, H], F32, tag="wraw")
                            nc.sync.dma_start(
                                out=raw[:om],
                                in_=c_d[name][
                                    t,
                                    oc * 128 : oc * 128 + om,
                                    ci * 128 : (ci + 1) * 128,
                                    :,
                                ],
                            )
                            for h in range(H):
                                pt = pst.tile([128, 128], F32, tag="ptr")
                                nc.tensor.transpose(
                                    pt[:, :om], raw[:om, :, h], ident[:om, :om]
                                )
                                st = wstage.tile([128, 128], F32, tag="wst")
                                nc.vector.tensor_copy(st[:, :om], pt[:, :om])
                                col = (t * H + h) * O + oc * 128
                                nc.sync.dma_start(
                                    out=scr[ci, :, col : col + om],
                                    in_=st[:, :om],
                                )

            # ---- feature load + transpose into X_T [128, 2, 2048]
            x_t = acts.tile([128, 4, NS], F32, tag="acts")
            for nt in range(NS // 128):
                fr = wraw.tile([128, 256], F32, tag="fraw")
                nc.sync.dma_start(out=fr, in_=f_d[nt * 128 : (nt + 1) * 128, :])
                for ic in range(2):
                    pt = pst.tile([128, 128], F32, tag="ptr")
                    nc.tensor.transpose(pt, fr[:, ic * 128 : (ic + 1) * 128], ident)
                    nc.vector.tensor_copy(x_t[:, ic, nt * 128 : (nt + 1) * 128], pt)

            # bias columns
            bias_cols = {}
            for bn, width in [("pred0_b", 512), ("pred1_b", 64)]:
                src = b_d[bn][:, :].rearrange("a (p c) -> (a p) c", c=1)
                for oc in range((width + 127) // 128):
                    om = min(128, width - oc * 128)
                    bt = singles.tile([om, 1], F32, name=f"bc_{bn}_{oc}")
                    nc.sync.dma_start(out=bt, in_=src[oc * 128 : oc * 128 + om])
                    bias_cols[(bn, oc)] = bt

            z_t = None
            cur = x_t
            out_t = None
            pred_t = None

            for name, I, O, bias in LAYERS:
                CI, OC = I // 128, (O + 127) // 128
                scr = wt_scr[name]
                src_t = z_t if name in ("dec_c", "pred0_c") else cur
                if name == "dec_c":
                    y_t = outsp.tile([128, 2, NS], F32, name="out_t")
                    out_t = y_t
                elif name == "pred1_c":
                    y_t = outsp.tile([64, 1, NS], F32, name="pred_t")
                    pred_t = y_t
                else:
                    y_t = acts.tile([128, 4, NS], F32, tag="acts")

                for nb in range(4):  # 512-wide batch-column chunks
                    n0 = nb * 512
                    psums = {}
                    for ci in range(CI):
                        wsb = win.tile([128, 2 * H * 512], F32, tag="win")
                        nc.sync.dma_start(
                            out=wsb[:, : 2 * H * O], in_=scr[ci]
                        )
                        xs = src_t[:, ci, n0 : n0 + 512]
                        # range reduce to [-pi, pi]
                        g = trigp.tile([128, 512], F32, tag="g")
                        nc.vector.tensor_scalar(g, xs, math.pi, None, OP.is_gt)
                        l = trigp.tile([128, 512], F32, tag="l")
                        nc.vector.tensor_scalar(l, xs, -math.pi, None, OP.is_lt)
                        d = trigp.tile([128, 512], F32, tag="d")
                        nc.vector.tensor_sub(d, g, l)
                        xt = trigp.tile([128, 512], F32, tag="xt")
                        nc.vector.scalar_tensor_tensor(
                            out=xt, in0=d, scalar=-TWO_PI, in1=xs,
                            op0=OP.mult, op1=OP.add,
                        )
                        # base trig: s1 = sin(xt), c1 = 1 - 2*sin(xt/2)^2
                        s1 = ring.tile([128, 512], F32, tag="s")
                        nc.scalar.activation(s1, xt, AF.Sin)
                        u = trigp.tile([128, 512], F32, tag="u")
                        nc.scalar.activation(u, xt, AF.Sin, scale=0.5)
                        u2 = trigp.tile([128, 512], F32, tag="u2")
                        nc.vector.tensor_mul(u2, u, u)
                        c1 = ring.tile([128, 512], F32, tag="c")
                        nc.vector.tensor_scalar(
                            c1, u2, -2.0, 1.0, OP.mult, OP.add
                        )
                        s_prev2 = c_prev2 = None
                        s_prev, c_prev = s1, c1
                        for k in range(1, H + 1):
                            if k == 1:
                                sk, ck = s1, c1
                            else:
                                sm = trigp.tile([128, 512], F32, tag="sm")
                                nc.vector.tensor_mul(sm, c1, s_prev)
                                sk = ring.tile([128, 512], F32, tag="s")
                                if k == 2:
                                    nc.vector.tensor_scalar(
                                        sk, sm, 2.0, None, OP.mult
                                    )
                                else:
                                    nc.vector.scalar_tensor_tensor(
                                        out=sk, in0=sm, scalar=2.0, in1=s_prev2,
                                        op0=OP.mult, op1=OP.subtract,
                                    )
                                cm = trigp.tile([128, 512], F32, tag="cm")
                                nc.vector.tensor_mul(cm, c1, c_prev)
                                ck = ring.tile([128, 512], F32, tag="c")
                                if k == 2:
                                    nc.vector.tensor_scalar(
                                        ck, cm, 2.0, -1.0, OP.mult, OP.add
                                    )
                                else:
                                    nc.vector.scalar_tensor_tensor(
                                        out=ck, in0=cm, scalar=2.0, in1=c_prev2,
                                        op0=OP.mult, op1=OP.subtract,
                                    )
                                s_prev2, s_prev = s_prev, sk
                                c_prev2, c_prev = c_prev, ck
                            for oc in range(OC):
                                om = min(128, O - oc * 128)
                                if oc not in psums:
                                    psums[oc] = psmm.tile(
                                        [om, 512], F32, tag=f"mm{oc}",
                                        name=f"ps_{name}_{nb}_{oc}",
                                    )
                                ps = psums[oc]
                                first = ci == 0 and k == 1
                                last = ci == CI - 1 and k == H
                                col0 = (0 * H + (k - 1)) * O + oc * 128
                                col1 = (1 * H + (k - 1)) * O + oc * 128
                                nc.tensor.matmul(
                                    ps, wsb[:, col0 : col0 + om], ck,
                                    start=first, stop=False,
                                )
                                nc.tensor.matmul(
                                    ps, wsb[:, col1 : col1 + om], sk,
                                    start=False, stop=last,
                                )
                    for oc, ps in psums.items():
                        om = min(128, O - oc * 128)
                        dst = y_t[:om, oc, n0 : n0 + 512]
                        if bias is not None:
                            nc.scalar.activation(
                                dst, ps, AF.Identity, bias=bias_cols[(bias, oc)]
                            )
                        else:
                            nc.scalar.copy(dst, ps)

                if name == "enc3_c":
                    z_t = y_t
                if name not in ("dec_c", "pred1_c"):
                    cur = y_t

            # ---- outputs: transpose back and store
            for nt in range(NS // 128):
                for oc2 in range(2):
                    pt = pst.tile([128, 128], F32, tag="ptr")
                    nc.tensor.transpose(
                        pt, out_t[:, oc2, nt * 128 : (nt + 1) * 128], ident
                    )
                    st = stagep.tile([128, 128], F32, tag="ost")
                    nc.vector.tensor_copy(st, pt)
                    nc.sync.dma_start(
                        out=out_d[
                            nt * 128 : (nt + 1) * 128, oc2 * 128 : (oc2 + 1) * 128
                        ],
                        in_=st,
                    )
            sg = outsp.tile([64, NS], F32, name="sig")
            nc.scalar.activation(sg, pred_t[:, 0, :], AF.Sigmoid)
            for nt in range(NS // 128):
                pt = pst.tile([128, 128], F32, tag="ptr")
                nc.tensor.transpose(
                    pt[:, :64], sg[:, nt * 128 : (nt + 1) * 128], ident[:64, :64]
                )
                st = stagep.tile([128, 64], F32, tag="pst2")
                nc.vector.tensor_copy(st, pt[:, :64])
                nc.sync.dma_start(out=pred_d[nt * 128 : (nt + 1) * 128, :], in_=st)

    _split_waits(nc)
    return nc


_CACHE = {}


def kernel(**inputs):
    if "nc" not in _CACHE:
        _CACHE["nc"] = _build()
    nc = _CACHE["nc"]
    feats = np.ascontiguousarray(np.asarray(inputs["features"], dtype=np.float32))
    rep = {
        k: np.ascontiguousarray(np.asarray(v, dtype=np.float32))
        for k, v in inputs.items()
        if k != "features"
    }
    in_maps = []
    for c in range(N_CORES):
        m = dict(rep)
        m["features"] = feats[c * NS : (c + 1) * NS]
        in_maps.append(m)
    res = run_bass_kernel_spmd(nc, in_maps, list(range(N_CORES))).results
    out = np.concatenate([r["out_o"] for r in res], axis=0)
    pred = np.concatenate([r["pred_o"] for r in res], axis=0)
    return out, pred
